# revision 1
# baseline (speedup 1.0000x reference)
"""Trainium2 Bass kernel for nn_CounterexampleGenerator (dense_mlp, memory-bound).

Strategy (8 NeuronCores, SPMD, no collectives):
  - Data-parallel over batch B=64: each core gets 8 batches = a contiguous
    [8192, 2048] f32 slice of x (64 MiB) — reading it once is the roofline.
  - Phase A (per core): stream x in 2 MiB tiles [128, 2, 2048]; one DVE add
    folds the two 128-row groups; PE column-sum matmuls (lhsT = tile chunk,
    rhs = ones) accumulate the L-reduction into PSUM, producing pooled in a
    TRANSPOSED layout pooledT[p, dc, b] = pooled[b, dc*128+p] (d on
    partitions) so the PGD loop needs no transposes at all.
  - Phase B: 10 PGD steps on [128, 16, 8] tiles. z1T = sum_dc W1c.T @ xaTc
    (W1 chunks stationary); u = W2 * gelu'(z1T + b1) via one ScalarE
    Derivative_Gelu (b1 is a per-partition bias in this layout) and one
    per-partition tensor_scalar_mul; gT chunks = W1Tc.T @ u (host-supplied
    W1.T chunks stationary); sign + clip updates on VectorE. The positive
    sigmoid' factor is dropped — it cannot change sign(grad).
  - Final score: Gelu, W2 matmul, Sigmoid; each core emits worst_score[1,8].
  - Host: gather 8x8 scores, cert = 1 - ws, violated = min(ws) < 0.1,
    x passes through untouched.
"""

import numpy as np

B, L, D, H = 64, 1024, 2048, 128
N_CORES = 8
BPC = B // N_CORES  # batches per core = 8
DC = D // 128  # 16 d-chunks
BUDGET = 10
STEP = 0.01
EPS = 0.2

# Phase-A tiling: each DMA tile holds LC row-groups of 128 rows x D cols.
LC = 2  # row-groups per tile -> [128, LC*2048] f32 = 2 MiB
TILES_PER_BATCH = (L // 128) // LC  # 4
N_TILES = BPC * TILES_PER_BATCH  # 32

_CACHE = {}


def _build_nc():
    import concourse.bacc as bacc
    import concourse.tile as tile
    import concourse.mybir as mybir

    f32 = mybir.dt.float32
    AF = mybir.ActivationFunctionType
    ALU = mybir.AluOpType

    nc = bacc.Bacc("TRN2", target_bir_lowering=False, debug=False)

    xs = nc.dram_tensor("xs", [BPC * L, D], f32, kind="ExternalInput")
    w1 = nc.dram_tensor("w1", [D, H], f32, kind="ExternalInput")
    w1t = nc.dram_tensor("w1t", [H, D], f32, kind="ExternalInput")
    w2 = nc.dram_tensor("w2", [H, 1], f32, kind="ExternalInput")
    b1 = nc.dram_tensor("b1", [H, 1], f32, kind="ExternalInput")
    b2 = nc.dram_tensor("b2", [1, 1], f32, kind="ExternalInput")
    noiset = nc.dram_tensor("noiset", [128, DC, BPC], f32, kind="ExternalInput")
    ws = nc.dram_tensor("ws", [1, BPC], f32, kind="ExternalOutput")

    # [t, p, lc, d] view of the x shard: partition p = row % 128.
    xview = xs.ap().rearrange("(t lc p) d -> t p lc d", lc=LC, p=128)

    with tile.TileContext(nc) as tc:
        with (
            tc.tile_pool(name="xin", bufs=6) as xin,
            tc.tile_pool(name="singles", bufs=1) as singles,
            tc.tile_pool(name="work", bufs=2) as work,
            tc.tile_pool(name="psA", bufs=2, space="PSUM") as psA,
            tc.tile_pool(name="psB", bufs=2, space="PSUM") as psB,
        ):
            # --- constants ---
            w1sb = singles.tile([128, DC, H], f32)
            nc.sync.dma_start(out=w1sb, in_=w1.ap().rearrange("(c p) h -> p c h", p=128))
            w1tsb = singles.tile([H, DC, 128], f32)
            nc.sync.dma_start(out=w1tsb, in_=w1t.ap().rearrange("h (c d) -> h c d", c=DC))
            w2sb = singles.tile([H, 1], f32)
            nc.sync.dma_start(out=w2sb, in_=w2.ap())
            b1sb = singles.tile([H, 1], f32)
            nc.sync.dma_start(out=b1sb, in_=b1.ap())
            b2sb = singles.tile([1, 1], f32)
            nc.sync.dma_start(out=b2sb, in_=b2.ap())
            noisesb = singles.tile([128, DC, BPC], f32)
            nc.sync.dma_start(out=noisesb, in_=noiset.ap())
            ones = singles.tile([128, 1], f32)
            nc.vector.memset(ones, 1.0)

            pooledT = singles.tile([128, DC, BPC], f32)
            xaT = singles.tile([128, DC, BPC], f32)

            # --- Phase A: pooled = mean_L(x), produced transposed ---
            psum_b = None
            for t in range(N_TILES):
                b = t // TILES_PER_BATCH
                tq = t % TILES_PER_BATCH
                xt = xin.tile([128, LC, D], f32)
                nc.sync.dma_start(out=xt, in_=xview[t])
                # fold the LC row-groups into group 0
                nc.vector.tensor_add(out=xt[:, 0, :], in0=xt[:, 0, :], in1=xt[:, 1, :])
                if tq == 0:
                    psum_b = psA.tile([128, DC], f32)
                for dc in range(DC):
                    nc.tensor.matmul(
                        psum_b[:, dc : dc + 1],
                        xt[:, 0, dc * 128 : (dc + 1) * 128],
                        ones,
                        start=(tq == 0),
                        stop=(tq == TILES_PER_BATCH - 1),
                    )
                if tq == TILES_PER_BATCH - 1:
                    # pooledT[:, :, b] = psum_b / L
                    nc.scalar.mul(out=pooledT[:, :, b], in_=psum_b, mul=1.0 / L)

            # x_adv0 = pooled + 0.01 * noise
            nc.vector.scalar_tensor_tensor(
                out=xaT, in0=noisesb, scalar=0.01, in1=pooledT,
                op0=ALU.mult, op1=ALU.add,
            )

            # --- Phase B: PGD ---
            for step in range(BUDGET):
                psz1 = psB.tile([H, BPC], f32)
                for dc in range(DC):
                    nc.tensor.matmul(
                        psz1,
                        w1sb[:, dc, :],
                        xaT[:, dc, :],
                        start=(dc == 0),
                        stop=(dc == DC - 1),
                    )
                # u = W2 * gelu'(z1 + b1)   (sigmoid' > 0 dropped: sign-invariant)
                u = work.tile([H, BPC], f32)
                nc.scalar.activation(
                    out=u, in_=psz1, func=AF.Derivative_Gelu, bias=b1sb, scale=1.0
                )
                nc.vector.tensor_scalar_mul(out=u, in0=u, scalar1=w2sb)

                psg = psB.tile([128, DC, BPC], f32)
                for dc in range(DC):
                    nc.tensor.matmul(
                        psg[:, dc, :], w1tsb[:, dc, :], u, start=True, stop=True
                    )
                sgn = work.tile([128, DC, BPC], f32)
                nc.scalar.sign(out=sgn, in_=psg)
                # xa' = xa - STEP*sgn ; xa = pooled + clip(xa' - pooled, +-EPS)
                delta = work.tile([128, DC, BPC], f32)
                nc.vector.scalar_tensor_tensor(
                    out=delta, in0=sgn, scalar=-STEP, in1=xaT,
                    op0=ALU.mult, op1=ALU.add,
                )
                nc.vector.tensor_sub(out=delta, in0=delta, in1=pooledT)
                nc.vector.tensor_scalar(
                    out=delta, in0=delta, scalar1=-EPS, scalar2=EPS,
                    op0=ALU.max, op1=ALU.min,
                )
                nc.vector.tensor_add(out=xaT, in0=delta, in1=pooledT)

            # --- final score ---
            psz1 = psB.tile([H, BPC], f32)
            for dc in range(DC):
                nc.tensor.matmul(
                    psz1,
                    w1sb[:, dc, :],
                    xaT[:, dc, :],
                    start=(dc == 0),
                    stop=(dc == DC - 1),
                )
            hT = work.tile([H, BPC], f32)
            nc.scalar.activation(out=hT, in_=psz1, func=AF.Gelu, bias=b1sb, scale=1.0)
            psz2 = psB.tile([1, BPC], f32)
            nc.tensor.matmul(psz2, w2sb, hT, start=True, stop=True)
            s_sb = work.tile([1, BPC], f32)
            nc.scalar.activation(
                out=s_sb, in_=psz2, func=AF.Sigmoid, bias=b2sb, scale=1.0
            )
            nc.sync.dma_start(out=ws.ap(), in_=s_sb)

    nc.compile()
    return nc


def _get_nc():
    if "nc" not in _CACHE:
        _CACHE["nc"] = _build_nc()
    return _CACHE["nc"]


def _noise_host():
    """noise = jax.random.normal(key(1), (B, D), f32), computed on host CPU."""
    if "noise" not in _CACHE:
        import jax
        import jax.numpy as jnp

        cpu = jax.devices("cpu")[0]
        with jax.default_device(cpu):
            key = jax.random.key(1)
            _CACHE["noise"] = np.asarray(
                jax.random.normal(key, (B, D), dtype=jnp.float32)
            )
    return _CACHE["noise"]


def _in_maps(x, W1, b1, W2, b2):
    noise = _noise_host()
    w1 = np.ascontiguousarray(W1, dtype=np.float32)
    w1t = np.ascontiguousarray(W1.T, dtype=np.float32)
    w2 = np.ascontiguousarray(W2, dtype=np.float32).reshape(H, 1)
    b1r = np.ascontiguousarray(b1, dtype=np.float32).reshape(H, 1)
    b2r = np.ascontiguousarray(b2, dtype=np.float32).reshape(1, 1)
    maps = []
    for c in range(N_CORES):
        xs = np.ascontiguousarray(x[c * BPC : (c + 1) * BPC]).reshape(BPC * L, D)
        nslice = noise[c * BPC : (c + 1) * BPC]  # [8, 2048]
        noiset = np.ascontiguousarray(
            nslice.reshape(BPC, DC, 128).transpose(2, 1, 0)
        )  # [128, 16, 8]
        maps.append(
            {
                "xs": xs,
                "w1": w1,
                "w1t": w1t,
                "w2": w2,
                "b1": b1r,
                "b2": b2r,
                "noiset": noiset,
            }
        )
    return maps


def run_device(x, W1, b1, W2, b2):
    """Run the Bass kernel on 8 cores; returns worst_score [B] f32 and the
    raw BassKernelResults (for timing introspection)."""
    from concourse.bass_utils import run_bass_kernel_spmd

    nc = _get_nc()
    maps = _in_maps(x, W1, b1, W2, b2)
    res = run_bass_kernel_spmd(nc, maps, core_ids=list(range(N_CORES)))
    ws = np.concatenate([res.results[c]["ws"][0] for c in range(N_CORES)])
    return ws.astype(np.float32), res


def kernel(x, W1, b1, W2, b2):
    x = np.asarray(x)
    worst_score, _ = run_device(
        x,
        np.asarray(W1),
        np.asarray(b1),
        np.asarray(W2),
        np.asarray(b2),
    )
    cert_score = (np.float32(1.0) - worst_score).astype(np.float32)
    violated = np.bool_(worst_score.min() < np.float32(0.1))
    return x, worst_score, cert_score, violated
